# revision 20
# baseline (speedup 1.0000x reference)
"""Batched sparse forward projection Y[b,r] = sum_k vals[k]*X[b,cols[k]] for rows[k]==r.

Strategy (8 NeuronCores, row-sharded):
- Each core owns a 16384-row slice; nnz slice via searchsorted (rows sorted).
- nnz bucketed by col>>13 into 8 buckets = 8 GPSIMD Q7 cores; stable bucketing
  keeps rows sorted per bucket.
- Gather via ap_gather: X table [128ch, 8192, 1] f32, channel 16c+j (j<8) holds
  X[j, 8192c+e]; per-Q7-core wrapped int16 index lists fetch all 8 batch values
  per nnz.
- Per chunk (768 output rows): contrib = gathered * vals (DVE), then a plain
  free-dim cumsum via tensor_tensor_scan, then a second ap_gather extracts the
  cumsum at each row's last-slot position (ends list, with a leading zero-slot);
  adjacent diffs (bf16) land in a persistent dt_all buffer.
- After all chunks: a batched matmul phase ([128,8] bf16 selection against
  dt_all slices) accumulates buckets into PSUM and DMAs straight to HBM.

This revision minimizes instruction/semaphore count (the dominant cost on this
runtime: ~130-200us per cross-engine dependency edge): one merged metadata DMA
per chunk (gather idx + extraction idx), one 3-dim-AP vals DMA per chunk
(replaces 8 per-bucket DMAs), and the matmul/output stage hoisted out of the
chunk loop entirely.
"""

import ml_dtypes
import numpy as np

import concourse.bass as bass
import concourse.mybir as mybir
import concourse.tile as tile
from concourse import bacc
from concourse.bass_utils import run_bass_kernel_spmd

B = 8
N_PIX = 65536
N_ROWS = 131072
N_CORES = 8
NBUK = 8
BUK = N_PIX // NBUK  # 8192
P = 128
RPC = 1184  # rows per chunk

_compiled = {}


def _ceil_to(x, m):
    return -(-x // m) * m


def _prep_core(rows_l, cols_n, vals_n, rows_per_core, rpc):
    """Sort by (bucket, row); per-(bucket,chunk) slot needs (+1 zero slot)."""
    buk = (cols_n >> 13).astype(np.int64)
    e = (cols_n & (BUK - 1)).astype(np.int16)
    key = buk * rows_per_core + rows_l.astype(np.int64)
    perm = np.argsort(key, kind="stable")
    skey = key[perm]
    cnt = np.bincount(key, minlength=NBUK * rows_per_core).reshape(NBUK, rows_per_core)
    n_chunks = -(-rows_per_core // rpc)
    need = np.zeros((NBUK, n_chunks), np.int64)
    for k in range(n_chunks):
        r0, r1 = k * rpc, min((k + 1) * rpc, rows_per_core)
        need[:, k] = cnt[:, r0:r1].sum(axis=1) + 1  # +1 zero slot
    return {
        "perm": perm,
        "skey": skey,
        "cnt": cnt,
        "e": e,
        "vals": vals_n,
        "need": need,
        "n_chunks": n_chunks,
    }


def _layout_core(prep, cls_, rows_per_core, rpc):
    n_chunks = len(cls_)
    Ltot = int(sum(cls_))
    cnt = prep["cnt"]
    cbase = np.concatenate([[0], np.cumsum(cls_)]).astype(np.int64)

    skey, perm = prep["skey"], prep["perm"]
    c_sorted = skey // rows_per_core
    r_sorted = skey % rows_per_core
    chunk_id = r_sorted // rpc
    seg_key = c_sorted * n_chunks + chunk_id
    seg_cnt = np.bincount(seg_key, minlength=NBUK * n_chunks)
    seg_start = np.cumsum(seg_cnt) - seg_cnt
    rank = np.arange(skey.shape[0], dtype=np.int64) - seg_start[seg_key]
    pos = cbase[chunk_id] + 1 + rank  # +1 for the zero slot

    idx16 = np.full((NBUK, Ltot), -1, np.int16)
    valsd = np.zeros((NBUK, Ltot), np.float32)
    idx16[c_sorted, pos] = prep["e"][perm]
    valsd[c_sorted, pos] = prep["vals"][perm]

    # extraction lists per chunk: [0, ends(r0), ends(r0+1), ...] padded to rpc+16
    epl = rpc + 16
    epx = np.zeros((NBUK, n_chunks, epl), np.int16)
    ccnt = np.cumsum(cnt, axis=1)
    for k in range(n_chunks):
        r0, r1 = k * rpc, min((k + 1) * rpc, rows_per_core)
        prev = ccnt[:, r0 - 1] if r0 > 0 else np.zeros(NBUK, np.int64)
        ends = ccnt[:, r0:r1] - prev[:, None]  # last-slot pos (1-based w/ zero slot)
        epx[:, k, 1 : 1 + (r1 - r0)] = ends.astype(np.int16)
        epx[:, k, 1 + (r1 - r0) :] = ends[:, -1:].astype(np.int16)

    idxw = np.ascontiguousarray(
        idx16.reshape(NBUK, Ltot // 16, 16).transpose(0, 2, 1)
    ).reshape(NBUK * 16, Ltot // 16)
    epxw = np.ascontiguousarray(
        epx.reshape(NBUK, n_chunks * epl // 16, 16).transpose(0, 2, 1)
    ).reshape(NBUK * 16, n_chunks * epl // 16)

    # merged per-chunk metadata: [gather idx | extraction idx] per chunk
    mdw = np.zeros((P, (Ltot + n_chunks * epl) // 16), np.int16)
    off = 0
    for k in range(n_chunks):
        CL = int(cls_[k])
        s16 = int(cbase[k]) // 16
        mdw[:, off : off + CL // 16] = idxw[:, s16 : s16 + CL // 16]
        mdw[:, off + CL // 16 : off + (CL + epl) // 16] = epxw[
            :, k * epl // 16 : (k + 1) * epl // 16
        ]
        off += (CL + epl) // 16
    return {"mdw": mdw, "valsd": valsd}


def _build_nc(cls_, rpc, rows_per_core, repeat=1):
    n_chunks = len(cls_)
    Ltot = int(sum(cls_))
    epl = rpc + 16
    nc = bacc.Bacc("TRN2", target_bir_lowering=False, debug=False)
    f32, i16, bf16 = mybir.dt.float32, mybir.dt.int16, mybir.dt.bfloat16

    xt = nc.dram_tensor("xt", [P, BUK], f32, kind="ExternalInput")
    sel = nc.dram_tensor("sel", [P, B], bf16, kind="ExternalInput")
    mdw = nc.dram_tensor(
        "mdw", [P, (Ltot + n_chunks * epl) // 16], i16, kind="ExternalInput"
    )
    valsd = nc.dram_tensor("valsd", [NBUK, Ltot], bf16, kind="ExternalInput")
    y = nc.dram_tensor("y", [B, rows_per_core], f32, kind="ExternalOutput")

    sbases = np.concatenate([[0], np.cumsum([int(c) for c in cls_])])

    with tile.TileContext(nc) as tc:
        with (
            tc.tile_pool(name="tabp", bufs=1) as tabp,
            tc.tile_pool(name="selp", bufs=1) as selp,
            tc.tile_pool(name="onep", bufs=1) as onep,
            tc.tile_pool(name="mdp", bufs=2) as mdp,
            tc.tile_pool(name="gtp", bufs=1) as gtp,
            tc.tile_pool(name="valp", bufs=1) as valp,
            tc.tile_pool(name="etp", bufs=1) as etp,
            tc.tile_pool(name="dtap", bufs=1) as dtap,
            tc.tile_pool(name="ysbp", bufs=2) as ysbp,
            tc.tile_pool(name="psp", bufs=2, space="PSUM") as psp,
        ):
            tab_t = tabp.tile([P, BUK, 1], f32)
            nc.sync.dma_start(tab_t[:, :, 0], xt[:])
            sel_t = selp.tile([P, B], bf16)
            nc.sync.dma_start(sel_t[:], sel[:])
            ones_t = onep.tile([P, 1], f32)
            nc.vector.memset(ones_t[:], 1.0)
            dt_all = dtap.tile([P, rows_per_core], bf16)

            for _rep in range(repeat):
                mdoff = 0
                for k in range(n_chunks):
                    CL = int(cls_[k])
                    sbase = int(sbases[k])
                    rb = k * rpc
                    rpck = min(rpc, rows_per_core - rb)
                    mlen = (CL + epl) // 16

                    md = mdp.tile([P, mlen], i16, name="md", tag="md")
                    nc.sync.dma_start(md[:], mdw[:, mdoff : mdoff + mlen])
                    gt = gtp.tile([P, CL, 1], f32, name="gt", tag="gt")
                    nc.gpsimd.ap_gather(
                        out_ap=gt[:],
                        in_ap=tab_t[:],
                        idxs_ap=md[:, : CL // 16],
                        channels=P,
                        num_elems=BUK,
                        d=1,
                        num_idxs=CL,
                    )
                    vt = valp.tile([P, CL], bf16, name="vt", tag="val")
                    nc.sync.dma_start(
                        vt[:], bass.AP(valsd, sbase, [[Ltot, 8], [0, 16], [1, CL]])
                    )
                    nc.vector.tensor_tensor(
                        out=gt[:, :, 0],
                        in0=gt[:, :, 0],
                        in1=vt[:],
                        op=mybir.AluOpType.mult,
                    )
                    # plain inclusive cumsum along the chunk (per partition)
                    nc.vector.tensor_tensor_scan(
                        out=gt[:, :, 0],
                        data0=ones_t[:].to_broadcast([P, CL]),
                        data1=gt[:, :, 0],
                        initial=0.0,
                        op0=mybir.AluOpType.mult,
                        op1=mybir.AluOpType.add,
                    )
                    # extract cumsum at [0, end(r0), end(r0+1), ...]
                    et = etp.tile([P, epl, 1], f32, name="et", tag="et")
                    nc.gpsimd.ap_gather(
                        out_ap=et[:],
                        in_ap=gt[:],
                        idxs_ap=md[:, CL // 16 :],
                        channels=P,
                        num_elems=CL,
                        d=1,
                        num_idxs=epl,
                    )
                    nc.vector.tensor_tensor(
                        out=dt_all[:, rb : rb + rpck],
                        in0=et[:, 1 : rpck + 1, 0],
                        in1=et[:, 0:rpck, 0],
                        op=mybir.AluOpType.subtract,
                    )
                    mdoff += mlen

                # batched reduction: bucket-sum via PE, PSUM -> SBUF -> HBM
                for r0 in range(0, rows_per_core, 2048):
                    rl = min(2048, rows_per_core - r0)
                    ps = psp.tile([B, 2048], f32, name="ps", tag="ps")
                    for m in range(_ceil_to(rl, 512) // 512):
                        a, b_ = m * 512, min((m + 1) * 512, rl)
                        nc.tensor.matmul(
                            out=ps[:, a:b_],
                            lhsT=sel_t[:],
                            rhs=dt_all[:, r0 + a : r0 + b_],
                            start=True,
                            stop=True,
                        )
                    ysb = ysbp.tile([B, 2048], f32, name="ysb", tag="ysb")
                    nc.vector.tensor_copy(out=ysb[:, :rl], in_=ps[:, :rl])
                    nc.sync.dma_start(y[:, r0 : r0 + rl], ysb[:, :rl])
    nc.compile()
    return nc


def _full_prep(X, vals, rows, cols, rows_per_core, rpc, n_cores):
    n_chunks = -(-rows_per_core // rpc)
    bounds = np.searchsorted(rows, np.arange(n_cores + 1) * rows_per_core)
    preps = []
    for n in range(n_cores):
        k0, k1 = bounds[n], bounds[n + 1]
        preps.append(
            _prep_core(
                (rows[k0:k1] - n * rows_per_core).astype(np.int64),
                cols[k0:k1].astype(np.int64),
                vals[k0:k1],
                rows_per_core,
                rpc,
            )
        )
    need = np.stack([p["need"] for p in preps])
    cls_ = [int(_ceil_to(int(need[:, :, k].max()), 64)) for k in range(n_chunks)]
    assert max(cls_) <= 20480, f"chunk too big: {max(cls_)}"

    T = np.zeros((P, BUK), np.float32)
    for c in range(NBUK):
        T[16 * c : 16 * c + 8, :] = X[:, BUK * c : BUK * (c + 1)]
    selm = np.zeros((P, B), np.float32)
    for c in range(NBUK):
        for j in range(B):
            selm[16 * c + j, j] = 1.0
    selm = selm.astype(ml_dtypes.bfloat16)

    in_maps = []
    for n in range(n_cores):
        lay = _layout_core(preps[n], cls_, rows_per_core, rpc)
        in_maps.append(
            {
                "xt": T,
                "sel": selm,
                "mdw": lay["mdw"],
                "valsd": lay["valsd"].astype(ml_dtypes.bfloat16),
            }
        )
    return cls_, in_maps


def kernel(X, vals, rows, cols):
    X = np.asarray(X, np.float32)
    vals = np.asarray(vals, np.float32)
    rows = np.asarray(rows, np.int64)
    cols = np.asarray(cols, np.int64)
    rows_per_core = N_ROWS // N_CORES

    rpc = RPC
    while True:
        try:
            cls_, in_maps = _full_prep(X, vals, rows, cols, rows_per_core, rpc, N_CORES)
            break
        except AssertionError:
            rpc //= 2  # denser-than-expected chunks: halve rows per chunk
            if rpc < 64:
                raise
    key = (tuple(cls_), rpc, rows_per_core)
    if key not in _compiled:
        _compiled[key] = _build_nc(cls_, rpc, rows_per_core)
    nc = _compiled[key]
    res = run_bass_kernel_spmd(nc, in_maps, core_ids=list(range(N_CORES)))
    Y = np.concatenate([res.results[n]["y"] for n in range(N_CORES)], axis=1)
    return np.ascontiguousarray(Y, dtype=np.float32)
